# revision 56
# baseline (speedup 1.0000x reference)
"""Trainium2 Bass kernel for nn_AttentiveTransformer (TabNet attentive transformer).

Computes, for full inputs (N=16384, NA=256, F=2048):
    x  = a @ W.T + b
    xn = batchnorm(x)  (training mode, batch stats over all N rows)
    m  = sparsemax_ascending_variant(xn * ps)
    new_ps = ps * (1.5 - m)

Key identities used:
 * The reference "sparsemax" sorts ascending; its k_z condition at the last
   index is 1 + sum_f(s_max - s_f) > 0, always true, so k_z = D-1 EXACTLY and
   tau = (sum(z)+1)/(D-1), m = relu(z - tau). No sort needed.
 * BN stats from Gram partials: S1[f] = colsum(a).W_f, S2[f] = W_f^T G W_f,
   both linear in per-core contributions -> one tiny AllReduce.
   var = S2/N - (S1/N)^2; the affine normalization is folded into the matmul:
   W' = W*s, bias t = bn_b - (S1/N)*s (b cancels).
 * ps ships NEGATED in fp16 (psn = -ps): zt = px*psn = -z keeps the rowsum
   trick, and nps = (m - GAMMA)*psn fuses to ONE scalar_tensor_tensor.
 * Precision budget vs the 2e-2 gate: single-pass bf16 matmuls throughout,
   fp16 ps and fp16 outputs, bf16 stats folds (t_lo dropped; |t|~0.01).
   Total ~4e-3 rel worst case measured, ~5x margin.

Queue discipline (engine DMA queues drain in order, so an AllReduce-dependent
DMA would stall everything behind it):
 * SP (sync):     pure prefetch: a chunks, aT, bn, all 16 ps tiles; then the
                  ttl1 row gather (post-AR), m writes and odd nps writes.
 * Act (scalar):  wT loads first, srow evictions, cc_in (AR input), the s_lo
                  row gather, mt activations.
 * Pool (gpsimd): the AllReduce, st1/st2 loads, s_hi row gather, fused nps
                  products, even nps writes.
 * Stats math in [16,128] layout; row gathers are one [1,F]-dest DMA each
   (~3.2us in the cost model) and run on three different queues in parallel.

Sharding: data-parallel over rows, 2048 rows/core on 8 cores; a single 16KB
AllReduce merges the BN stats.

Set KLOOP=<n> to build the kernel body n times back-to-back in one NEFF
(wall-clock HW timing via (T(n) - T(1))/(n-1)).
"""

import os
import sys
import numpy as np

for _p in ("/opt/trn_rl_repo",):
    if _p not in sys.path:
        sys.path.insert(0, _p)

KLOOP = int(os.environ.get("KLOOP", "1"))
KVAR = os.environ.get("KVAR", "")             # debug variant flags

N, NA, F = 16384, 256, 2048
NCORES = 8
NSH = N // NCORES            # 2048 rows per core
P = 128                      # partitions
RT = NSH // P                # 16 row-tiles per core
FCW = 512                    # feature chunk width (psum bank / max moving free)
FC = F // FCW                # 4 feature chunks
FP = F // P                  # 16 (partition dim of the [16,128] stats layout)
NAUG = NA + 1                # 257: a with ones column (colsum rides the Gram)
GAMMA = 1.5
BN_EPS = 1e-5
INV_D1 = 1.0 / (F - 1.0)     # 1/2047

_CACHE = {}


def _build_bass(kloop=None):
    KLOOP = globals()["KLOOP"] if kloop is None else kloop
    import concourse.mybir as mybir
    import concourse.tile as tile
    from concourse import bacc
    from concourse.bass import ts

    fp32 = mybir.dt.float32
    fp16 = mybir.dt.float16
    bf16 = mybir.dt.bfloat16
    Alu = mybir.AluOpType
    Act = mybir.ActivationFunctionType

    nc = bacc.Bacc(
        "TRN2",
        target_bir_lowering=False,
        debug=False,
        enable_asserts=False,
        num_devices=NCORES,
    )

    ah_aug = nc.dram_tensor("ah_aug", [NSH, NAUG], bf16, kind="ExternalInput").ap()
    ahT = nc.dram_tensor("ahT", [NA, NSH], bf16, kind="ExternalInput").ap()
    wTh = nc.dram_tensor("wTh", [NA, F], bf16, kind="ExternalInput").ap()
    ps_in = nc.dram_tensor("ps_in", [NSH, F], fp16, kind="ExternalInput").ap()
    bnw16 = nc.dram_tensor("bnw16", [FP, P], fp32, kind="ExternalInput").ap()
    bnb16 = nc.dram_tensor("bnb16", [FP, P], fp32, kind="ExternalInput").ap()
    m_out = nc.dram_tensor("m_out", [NSH, F], fp16, kind="ExternalOutput").ap()
    nps_out = nc.dram_tensor("nps_out", [NSH, F], fp16, kind="ExternalOutput").ap()

    ps_t = ps_in.rearrange("(t p) f -> t p f", p=P)
    m_t = m_out.rearrange("(t p) f -> t p f", p=P)
    nps_t = nps_out.rearrange("(t p) f -> t p f", p=P)
    ah_j = ah_aug.rearrange("(j s p) c -> j p s c", p=P, s=4)

    with tile.TileContext(nc) as tc:
        with tc.tile_pool(name="res", bufs=1) as res, \
             tc.tile_pool(name="dram", bufs=1, space="DRAM") as dram:
          for it in range(KLOOP):
            # ps prefetch pool FIRST so its SBUF space never aliases released
            # prologue space (an aliased WAR dep would chain it to the AR).
            psb = tc.alloc_tile_pool(name=f"psb{it}", bufs=RT)
            pro = tc.alloc_tile_pool(name=f"pro{it}", bufs=1)

            # ---------------- constants ----------------
            ones_colb = pro.tile([P, 1], bf16)
            nc.vector.memset(ones_colb, 1.0)
            warm = pro.tile([1, 1], fp32)
            nc.vector.memset(warm, 1.0)

            # ---------------- phase 1: Gram partials (single bf16 pass) -----
            g0h = pro.tile([P, NA], bf16)
            g1h = pro.tile([P, NA], bf16)
            sc0h = pro.tile([P, 1], bf16)
            sc1h = pro.tile([P, 1], bf16)
            with tc.tile_pool(name=f"pro1{it}", bufs=1, space="PSUM") as pp1, \
                 tc.tile_pool(name=f"abig{it}", bufs=4) as abigp:
                pg0 = pp1.tile([P, NAUG], fp32)
                pg1 = pp1.tile([P, NAUG], fp32)
                for j in range(4):
                    hch = abigp.tile([P, 4, NAUG], bf16, name="hch")
                    nc.sync.dma_start(hch, ah_j[j])
                    for s in range(4):
                        first = j == 0 and s == 0
                        last = j == 3 and s == 3
                        ah_t = hch[:, s, :]
                        for half, pg in ((0, pg0), (1, pg1)):
                            nc.tensor.matmul(pg, ah_t[:, ts(half, P)], ah_t,
                                             start=first, stop=last)
                # W (pre-split bf16 on host) rides the Act queue right after
                # its two a-chunks; warm preloads the Sqrt table in Act's
                # pre-phase2 idle gap.
                w0hr = pro.tile([P, F], bf16)
                nc.scalar.dma_start(w0hr, wTh[0:P, :])
                w1hr = pro.tile([P, F], bf16)
                nc.scalar.dma_start(w1hr, wTh[P:NA, :])
                nc.scalar.activation(warm, warm, Act.Sqrt)
                for pg, gh, sch in ((pg0, g0h, sc0h), (pg1, g1h, sc1h)):
                    nc.vector.tensor_copy(gh, pg[:, 0:NA])
                    nc.vector.tensor_copy(sch, pg[:, NA:NAUG])

            # ---------------- resident loads + full ps prefetch (SP queue) ---
            ah0 = res.tile([P, NSH], bf16, name="ah0")
            nc.sync.dma_start(ah0, ahT[0:P, :])
            ah1 = res.tile([P, NSH], bf16, name="ah1")
            nc.sync.dma_start(ah1, ahT[P:NA, :])
            bnw_c = pro.tile([FP, P], fp32)
            nc.sync.dma_start(bnw_c, bnw16)
            bnb_c = pro.tile([FP, P], fp32)
            nc.sync.dma_start(bnb_c, bnb16)
            psts = []
            for rt in range(RT):
                pst = psb.tile([P, F], fp16, name="pst")
                nc.sync.dma_start(pst, ps_t[rt])
                psts.append(pst)

            # ---------------- phase 2: S1/S2 partials ----------------
            # H = G @ W^T in single bf16; S2 = colsum(H*W) via an fp32 ones
            # pass directly on qf (no bf16 eviction hop); S1 = colsum.W^T.
            srow = pro.tile([1, 2 * F], fp32)   # cols 0:F = S1 partial, F:2F = S2
            with tc.tile_pool(name=f"pro2{it}", bufs=1, space="PSUM") as pp2, \
                 tc.tile_pool(name=f"qtmp{it}", bufs=2) as qtmp:
                qfs = []
                def s2_pass(fc):
                    q0, q1 = qfs[fc]
                    fsl = ts(fc, FCW)
                    ps2 = pp2.tile([1, FCW], fp32, name="ps2", tag="ps2", bufs=2)
                    nc.tensor.matmul(ps2, ones_colb, q0, start=True, stop=False)
                    nc.tensor.matmul(ps2, ones_colb, q1, start=False, stop=True)
                    ps1 = pp2.tile([1, FCW], fp32, name="ps1", tag="ps1", bufs=2)
                    nc.tensor.matmul(ps1, sc0h, w0hr[:, fsl], start=True, stop=False)
                    nc.tensor.matmul(ps1, sc1h, w1hr[:, fsl], start=False, stop=True)
                    # (gpsimd cannot read PSUM on real HW; Act does these)
                    nc.scalar.copy(srow[0:1, fsl], ps1)
                    nc.scalar.copy(srow[0:1, ts(FC + fc, FCW)], ps2)

                for fc in range(FC):
                    fsl = ts(fc, FCW)
                    ph0 = pp2.tile([P, FCW], fp32, name="ph0", tag="ph0", bufs=2)
                    nc.tensor.matmul(ph0, g0h[:, 0:P], w0hr[:, fsl], start=True, stop=False)
                    nc.tensor.matmul(ph0, g1h[:, 0:P], w1hr[:, fsl], start=False, stop=True)
                    ph1 = pp2.tile([P, FCW], fp32, name="ph1", tag="ph1", bufs=2)
                    nc.tensor.matmul(ph1, g0h[:, P:NA], w0hr[:, fsl], start=True, stop=False)
                    nc.tensor.matmul(ph1, g1h[:, P:NA], w1hr[:, fsl], start=False, stop=True)
                    # DVE writes qf as bf16 directly: same rounding the old
                    # Act-copy hop applied, zero extra cost, 8 fewer Act ops
                    qf0 = qtmp.tile([P, FCW], bf16, name="qf0")
                    nc.vector.tensor_tensor(qf0, ph0, w0hr[:, fsl], Alu.mult)
                    qf1 = qtmp.tile([P, FCW], bf16, name="qf1")
                    nc.vector.tensor_tensor(qf1, ph1, w1hr[:, fsl], Alu.mult)
                    qfs.append((qf0, qf1))
                    if fc >= 1:
                        s2_pass(fc - 1)   # pipeline: PE stays a chunk ahead
                s2_pass(FC - 1)

            # ---------------- phase 3: AllReduce of S1,S2 (16KB) -------------
            # KVAR=noar: timing probe that skips the collective (stats become
            # single-core partials -> WRONG outputs, identical dense work).
            if "noar" not in KVAR:
                cc_in = dram.tile([1, 2 * F], fp32, name="cc_in")
                cc_out = dram.tile([1, 2 * F], fp32, addr_space="Shared", name="cc_out")
                nc.scalar.dma_start(cc_in, srow)
                nc.gpsimd.collective_compute(
                    "AllReduce",
                    Alu.add,
                    replica_groups=[list(range(NCORES))],
                    ins=[cc_in.opt()],
                    outs=[cc_out.opt()],
                )
                cc_r = cc_out.rearrange("o (two p c) -> two (o p) c", two=2, p=FP)
                st_src = [cc_r[0], cc_r[1]]
            else:
                cc_loc = dram.tile([1, 2 * F], fp32, name="cc_loc")
                nc.scalar.dma_start(cc_loc, srow)
                sr = cc_loc.rearrange("o (two p c) -> two (o p) c", two=2, p=FP)
                st_src = [sr[0], sr[1]]

            # ---------------- phase 4: stats math in [16,128] layout ---------
            shsl2 = res.tile([2, F], bf16, name="shsl2")  # rows: s hi, s lo
            ttl1 = res.tile([1, F], bf16, name="ttl1")    # t (bf16 only; |t|~.01)
            ones2 = res.tile([2, P], bf16, name="ones2")
            nc.vector.memset(ones2, 1.0)
            with tc.tile_pool(name=f"smath{it}", bufs=1) as sm:
                st12 = sm.tile([FP, 2 * P], fp32)
                nc.gpsimd.dma_start(st12[:, 0:P], st_src[0])
                nc.gpsimd.dma_start(st12[:, P:2 * P], st_src[1])
                st1 = st12[:, 0:P]
                st2 = st12[:, P:2 * P]
                # go token: unblocks the PE warmup dummies right after the AR
                go = res.tile([1, 1], bf16, name="go")
                nc.gpsimd.tensor_copy(go, st12[0:1, 0:1])
                sq = sm.tile([FP, P], fp32)
                nc.vector.tensor_tensor(sq, st1, st1, Alu.mult)
                # vv0 = S2 - S1^2/N  (= N*(var+eps) - N*eps)
                vv0 = sm.tile([FP, P], fp32)
                nc.vector.scalar_tensor_tensor(vv0, sq, -1.0 / N, st2, Alu.mult, Alu.add)
                # rr = sqrt(vv0 + N*eps): eps rides the activation bias
                epsb = sm.tile([FP, 1], fp32)
                nc.vector.memset(epsb, float(N * BN_EPS))
                rr = sm.tile([FP, P], fp32)
                nc.scalar.activation(rr, vv0, Act.Sqrt, bias=epsb)
                y0 = sm.tile([FP, P], fp32)
                nc.vector.reciprocal(y0, rr)
                # one Newton step for 1/sqrt(vv) (ScalarE Sqrt is low-precision):
                # y = y0*(1.5 - 0.5*vv*y0^2), with -0.5*vv prefolded
                yy = sm.tile([FP, P], fp32)
                nc.vector.tensor_tensor(yy, y0, y0, Alu.mult)
                nvv = sm.tile([FP, P], fp32)
                nc.vector.tensor_scalar(nvv, vv0, -0.5, -0.5 * N * BN_EPS, Alu.mult, Alu.add)
                u = sm.tile([FP, P], fp32)
                nc.vector.tensor_tensor(u, nvv, yy, Alu.mult)
                y = sm.tile([FP, P], fp32)
                nc.vector.scalar_tensor_tensor(y, u, 1.5, y0, Alu.add, Alu.mult)
                # s = sqrt(N) * y * bn_w; folded bias t = bn_b - (S1/N)*s.
                # s splits + gathers first: the W' fold chain (pbf -> w0h ->
                # px) is longer than the bias-row path, so s leaves earliest.
                s_c = sm.tile([FP, P], fp32)
                nc.vector.scalar_tensor_tensor(s_c, y, float(np.sqrt(N)), bnw_c, Alu.mult, Alu.mult)
                sh_c = sm.tile([FP, P], bf16)
                nc.vector.tensor_copy(sh_c, s_c)
                sl_c = sm.tile([FP, P], bf16)
                nc.vector.tensor_tensor(sl_c, s_c, sh_c, Alu.subtract)
                nc.gpsimd.dma_start(shsl2[0:1, :], sh_c)
                nc.scalar.dma_start(shsl2[1:2, :], sl_c)
                tm = sm.tile([FP, P], fp32)
                nc.vector.scalar_tensor_tensor(tm, st1, -1.0 / N, s_c, Alu.mult, Alu.mult)
                t_c = sm.tile([FP, P], fp32)
                nc.vector.tensor_tensor(t_c, tm, bnb_c, Alu.add)
                th_c = sm.tile([FP, P], bf16)
                nc.vector.tensor_copy(th_c, t_c)
                nc.sync.dma_start(ttl1, th_c)

            # ---------------- phase 5: fold scale into W^T (bf16 hi only) ----
            # PE warmup: the p-state ramp needs ~3us of continuous busy for
            # full clock. These dummy K=1 passes are gated on the AR result
            # (via the Pool-written go token) and run during the stats-math /
            # gather window, so pb/w0h/px hit the ramped clock.
            w0h = res.tile([P, F], bf16, name="w0h")
            w1h = res.tile([P, F], bf16, name="w1h")
            with tc.tile_pool(name=f"pro3{it}", bufs=2, space="PSUM") as pp3, \
                 tc.tile_pool(name=f"wsc{it}", bufs=2) as wsc:
                trash = pp3.tile([1, FCW], fp32, name="trash", tag="trash", bufs=1)
                for _ in range(8):
                    nc.tensor.matmul(trash, go, w0hr[0:1, 0:FCW], start=True, stop=True)
                # full-width s broadcast (PSUM), then ONE fold op per W half,
                # split DVE/Pool so they run in parallel
                pbf = pp3.tile([P, F], fp32, name="pbf", tag="pbf", bufs=1)
                for fc in range(FC):
                    nc.tensor.matmul(pbf[:, ts(fc, FCW)], ones2, shsl2[:, ts(fc, FCW)],
                                     start=True, stop=True)
                # both on DVE (gpsimd cannot read PSUM); px pass1 only needs
                # w0h, so the w1h fold overlaps the first px chunks
                nc.vector.tensor_tensor(w0h, w0hr, pbf, Alu.mult)
                nc.vector.tensor_tensor(w1h, w1hr, pbf, Alu.mult)
                # bridge the fold window too: any PE idle gap resets the ramp
                for _ in range(12):
                    nc.tensor.matmul(trash, go, w0hr[0:1, 0:FCW], start=True, stop=True)

            # ---------------- main loop over 16 row-tiles ----------------
            ones1 = res.tile([1, P], bf16, name="ones1")
            nc.vector.memset(ones1, 1.0)
            with tc.tile_pool(name=f"mx{it}", bufs=8, space="PSUM") as mxp, \
                 tc.tile_pool(name=f"zb{it}", bufs=2) as zb, \
                 tc.tile_pool(name=f"mb{it}", bufs=3) as mb, \
                 tc.tile_pool(name=f"ub{it}", bufs=3) as ub, \
                 tc.tile_pool(name=f"nb{it}", bufs=3) as nb, \
                 tc.tile_pool(name=f"rsb{it}", bufs=4) as rsb:
                for rt in range(RT):
                    rsl = ts(rt, P)
                    pst = psts[rt]   # psn = -ps (negated on host)
                    px = mxp.tile([P, F], fp32, name="px", tag="px", bufs=2)
                    ptypes = [(ah0[:, rsl], w0h), (ah1[:, rsl], w1h),
                              (ones1, ttl1)]
                    for pi, (lhsT, rhs) in enumerate(ptypes):
                        for fc in range(FC):
                            nc.tensor.matmul(px[:, ts(fc, FCW)], lhsT, rhs[:, ts(fc, FCW)],
                                             start=(pi == 0), stop=(pi == len(ptypes) - 1))
                    # zt = xn*psn = -z; rs = rowsum(zt) = -sum(z)
                    zt = zb.tile([P, F], fp32, name="zt")
                    rs = rsb.tile([P, 1], fp32, name="rs")
                    nc.vector.scalar_tensor_tensor(
                        zt, px, 1.0, pst, Alu.mult, Alu.mult, accum_out=rs,
                    )
                    # tau = (sum(z)+1)/2047; ntau = -tau
                    ntau = rsb.tile([P, 1], fp32, name="ntau")
                    nc.vector.tensor_scalar(ntau, rs, INV_D1, -INV_D1, Alu.mult, Alu.add)
                    # m = relu(z - tau) = relu(-zt + ntau)  [fp16 out]
                    mt = mb.tile([P, F], fp16, name="mt")
                    nc.scalar.activation(mt, zt, Act.Relu, bias=ntau, scale=-1.0)
                    nc.sync.dma_start(m_t[rt], mt)
                    # nps = ps*(GAMMA - m) = (m - GAMMA)*psn.
                    # (Pool only supports TensorTensor, so the -GAMMA shift
                    # runs elsewhere: DVE 16-bit mode is cheapest (594ns) but
                    # DVE is the steady cap, so ~6/16 tiles use Act's idle.)
                    ut = ub.tile([P, F], fp16, name="ut")
                    nc.vector.tensor_scalar_sub(ut, mt, GAMMA)
                    nt = nb.tile([P, F], fp16, name="nt")
                    nc.gpsimd.tensor_tensor(nt, ut, pst, Alu.mult)
                    # parity split balances SP/Pool; swap for the final pair so
                    # the tail m15/nps15 writes land on different queues
                    on_pool = rt % 2 == 0 if rt < RT - 2 else rt % 2 == 1
                    if on_pool:
                        nc.gpsimd.dma_start(nps_t[rt], nt)
                    else:
                        nc.sync.dma_start(nps_t[rt], nt)
            pro.release()
            psb.release()
            if "bar" in KVAR and it < KLOOP - 1:
                # cross-iteration serializer for K-loop timing probes: a tiny
                # AllReduce whose input depends on the last output tile
                bin_ = dram.tile([1, 8], fp16, name="bar_in")
                bout = dram.tile([1, 8], fp16, addr_space="Shared", name="bar_out")
                nc.sync.dma_start(bin_, nt[0:1, 0:8])
                nc.gpsimd.collective_compute(
                    "AllReduce",
                    Alu.add,
                    replica_groups=[list(range(NCORES))],
                    ins=[bin_.opt()],
                    outs=[bout.opt()],
                )
                bs = res.tile([1, 8], fp16, name="bar_s")
                # on the SP queue: stalls it until the barrier completes, so
                # the next iteration's prefetch cannot start early
                nc.sync.dma_start(bs, bout)

    nc.compile()
    return nc


def _get_nc(kloop=None):
    key = ("nc", KLOOP if kloop is None else kloop)
    if key not in _CACHE:
        _CACHE[key] = _build_bass(kloop=key[1])
    return _CACHE[key]


def _make_in_maps(a, ps, W, b, bn_w, bn_b):
    import ml_dtypes
    a = np.ascontiguousarray(a, dtype=np.float32)
    ah = a.astype(ml_dtypes.bfloat16)
    psn16 = np.ascontiguousarray((-ps).astype(np.float16))
    wTh_np = np.ascontiguousarray(W.astype(np.float32).T.astype(ml_dtypes.bfloat16))
    bnw16 = np.ascontiguousarray(bn_w.astype(np.float32).reshape(FP, P))
    bnb16 = np.ascontiguousarray(bn_b.astype(np.float32).reshape(FP, P))
    in_maps = []
    for c in range(NCORES):
        rows = slice(c * NSH, (c + 1) * NSH)
        ah_c = ah[rows]
        ah_aug = np.concatenate([ah_c, np.ones((NSH, 1), ah.dtype)], axis=1)
        in_maps.append({
            "ah_aug": np.ascontiguousarray(ah_aug),
            "ahT": np.ascontiguousarray(ah_c.T),
            "wTh": wTh_np,
            "ps_in": np.ascontiguousarray(psn16[rows]),
            "bnw16": bnw16,
            "bnb16": bnb16,
        })
    return in_maps


def run(a, ps, W, b, bn_w, bn_b, trace=False, **kw):
    """Run the kernel on the 8 NeuronCores; returns ((m, new_ps), BassKernelResults)."""
    from concourse import bass_utils

    nc = _get_nc()
    in_maps = _make_in_maps(a, ps, W, b, bn_w, bn_b)
    res = bass_utils.run_bass_kernel_spmd(
        nc, in_maps, core_ids=list(range(NCORES)), trace=trace, **kw,
    )
    m = np.concatenate([r["m_out"] for r in res.results], axis=0).astype(np.float32)
    nps = np.concatenate([r["nps_out"] for r in res.results], axis=0).astype(np.float32)
    return (m, nps), res


def kernel(a, ps, W, b, bn_w, bn_b):
    (m, nps), _ = run(a, ps, W, b, bn_w, bn_b, trace=False)
    return m, nps


if __name__ == "__main__":
    rng = np.random.default_rng(0)
    a = rng.standard_normal((N, NA), dtype=np.float32)
    ps = rng.random((N, F), dtype=np.float32)
    lim = 1.0 / np.sqrt(NA)
    W = rng.uniform(-lim, lim, (F, NA)).astype(np.float32)
    b = rng.uniform(-lim, lim, (F,)).astype(np.float32)
    bn_w = np.ones((F,), np.float32)
    bn_b = np.zeros((F,), np.float32)
    (m, nps), res = run(a, ps, W, b, bn_w, bn_b)
    print("m", m.shape, m.dtype, "nps", nps.shape)
    print("exec_time_ns:", res.exec_time_ns)


# revision 59
# speedup vs baseline: 1.0132x; 1.0132x over previous
"""Trainium2 Bass kernel for nn_AttentiveTransformer (TabNet attentive transformer).

Computes, for full inputs (N=16384, NA=256, F=2048):
    x  = a @ W.T + b
    xn = batchnorm(x)  (training mode, batch stats over all N rows)
    m  = sparsemax_ascending_variant(xn * ps)
    new_ps = ps * (1.5 - m)

Key identities used:
 * The reference "sparsemax" sorts ascending; its k_z condition at the last
   index is 1 + sum_f(s_max - s_f) > 0, always true, so k_z = D-1 EXACTLY and
   tau = (sum(z)+1)/(D-1), m = relu(z - tau). No sort needed.
 * BN stats from Gram partials: S1[f] = colsum(a).W_f, S2[f] = W_f^T G W_f,
   both linear in per-core contributions -> one tiny AllReduce.
   var = S2/N - (S1/N)^2; the affine normalization is folded into the matmul:
   W' = W*s, bias t = bn_b - (S1/N)*s (b cancels).
 * ps ships NEGATED in fp16 (psn = -ps): zt = px*psn = -z keeps the rowsum
   trick, and nps = (m - GAMMA)*psn fuses to ONE scalar_tensor_tensor.
 * Precision budget vs the 2e-2 gate: single-pass bf16 matmuls throughout,
   fp16 ps and fp16 outputs, bf16 stats folds (t_lo dropped; |t|~0.01).
   Total ~4e-3 rel worst case measured, ~5x margin.

Queue discipline (engine DMA queues drain in order, so an AllReduce-dependent
DMA would stall everything behind it):
 * SP (sync):     pure prefetch: a chunks, aT, bn, all 16 ps tiles; then the
                  ttl1 row gather (post-AR), m writes and odd nps writes.
 * Act (scalar):  wT loads first, srow evictions, cc_in (AR input), the s_lo
                  row gather, mt activations.
 * Pool (gpsimd): the AllReduce, st1/st2 loads, s_hi row gather, fused nps
                  products, even nps writes.
 * Stats math in [16,128] layout; row gathers are one [1,F]-dest DMA each
   (~3.2us in the cost model) and run on three different queues in parallel.

Sharding: data-parallel over rows, 2048 rows/core on 8 cores; a single 16KB
AllReduce merges the BN stats.

Set KLOOP=<n> to build the kernel body n times back-to-back in one NEFF
(wall-clock HW timing via (T(n) - T(1))/(n-1)).
"""

import os
import sys
import numpy as np

for _p in ("/opt/trn_rl_repo",):
    if _p not in sys.path:
        sys.path.insert(0, _p)

KLOOP = int(os.environ.get("KLOOP", "1"))
KVAR = os.environ.get("KVAR", "")             # debug variant flags

N, NA, F = 16384, 256, 2048
NCORES = 8
NSH = N // NCORES            # 2048 rows per core
P = 128                      # partitions
RT = NSH // P                # 16 row-tiles per core
FCW = 512                    # feature chunk width (psum bank / max moving free)
FC = F // FCW                # 4 feature chunks
FP = F // P                  # 16 (partition dim of the [16,128] stats layout)
NAUG = NA + 1                # 257: a with ones column (colsum rides the Gram)
GAMMA = 1.5
BN_EPS = 1e-5
INV_D1 = 1.0 / (F - 1.0)     # 1/2047

_CACHE = {}


def _build_bass(kloop=None):
    KLOOP = globals()["KLOOP"] if kloop is None else kloop
    import concourse.mybir as mybir
    import concourse.tile as tile
    from concourse import bacc
    from concourse.bass import ts

    fp32 = mybir.dt.float32
    fp16 = mybir.dt.float16
    bf16 = mybir.dt.bfloat16
    Alu = mybir.AluOpType
    Act = mybir.ActivationFunctionType

    nc = bacc.Bacc(
        "TRN2",
        target_bir_lowering=False,
        debug=False,
        enable_asserts=False,
        num_devices=NCORES,
    )

    ah_aug = nc.dram_tensor("ah_aug", [NSH, NAUG], bf16, kind="ExternalInput").ap()
    ahT = nc.dram_tensor("ahT", [NA, NSH], bf16, kind="ExternalInput").ap()
    wTh = nc.dram_tensor("wTh", [NA, F], bf16, kind="ExternalInput").ap()
    ps_in = nc.dram_tensor("ps_in", [NSH, F], fp16, kind="ExternalInput").ap()
    bnw16 = nc.dram_tensor("bnw16", [FP, P], fp32, kind="ExternalInput").ap()
    bnb16 = nc.dram_tensor("bnb16", [FP, P], fp32, kind="ExternalInput").ap()
    m_out = nc.dram_tensor("m_out", [NSH, F], fp16, kind="ExternalOutput").ap()
    nps_out = nc.dram_tensor("nps_out", [NSH, F], fp16, kind="ExternalOutput").ap()

    ps_t = ps_in.rearrange("(t p) f -> t p f", p=P)
    m_t = m_out.rearrange("(t p) f -> t p f", p=P)
    nps_t = nps_out.rearrange("(t p) f -> t p f", p=P)
    ah_j = ah_aug.rearrange("(j s p) c -> j p s c", p=P, s=4)

    with tile.TileContext(nc) as tc:
        with tc.tile_pool(name="res", bufs=1) as res, \
             tc.tile_pool(name="dram", bufs=1, space="DRAM") as dram:
          for it in range(KLOOP):
            # ps prefetch pool FIRST so its SBUF space never aliases released
            # prologue space (an aliased WAR dep would chain it to the AR).
            psb = tc.alloc_tile_pool(name=f"psb{it}", bufs=RT)
            pro = tc.alloc_tile_pool(name=f"pro{it}", bufs=1)

            # ---------------- constants ----------------
            ones_colb = pro.tile([P, 1], bf16)
            nc.vector.memset(ones_colb, 1.0)
            warm = pro.tile([1, 1], fp32)
            nc.vector.memset(warm, 1.0)

            # ---------------- phase 1: Gram partials (single bf16 pass) -----
            g0h = pro.tile([P, NA], bf16)
            g1h = pro.tile([P, NA], bf16)
            sc0h = pro.tile([P, 1], bf16)
            sc1h = pro.tile([P, 1], bf16)
            with tc.tile_pool(name=f"pro1{it}", bufs=1, space="PSUM") as pp1, \
                 tc.tile_pool(name=f"abig{it}", bufs=4) as abigp:
                pg0 = pp1.tile([P, NAUG], fp32)
                pg1 = pp1.tile([P, NAUG], fp32)
                for j in range(4):
                    hch = abigp.tile([P, 4, NAUG], bf16, name="hch")
                    nc.sync.dma_start(hch, ah_j[j])
                    for s in range(4):
                        first = j == 0 and s == 0
                        last = j == 3 and s == 3
                        ah_t = hch[:, s, :]
                        for half, pg in ((0, pg0), (1, pg1)):
                            nc.tensor.matmul(pg, ah_t[:, ts(half, P)], ah_t,
                                             start=first, stop=last)
                # W (pre-split bf16 on host) rides the Act queue right after
                # its two a-chunks; warm preloads the Sqrt table in Act's
                # pre-phase2 idle gap.
                w0hr = pro.tile([P, F], bf16)
                nc.scalar.dma_start(w0hr, wTh[0:P, :])
                w1hr = pro.tile([P, F], bf16)
                nc.scalar.dma_start(w1hr, wTh[P:NA, :])
                nc.scalar.activation(warm, warm, Act.Sqrt)
                for pg, gh, sch in ((pg0, g0h, sc0h), (pg1, g1h, sc1h)):
                    nc.vector.tensor_copy(gh, pg[:, 0:NA])
                    nc.vector.tensor_copy(sch, pg[:, NA:NAUG])

            # ---------------- resident loads + full ps prefetch (SP queue) ---
            ah0 = res.tile([P, NSH], bf16, name="ah0")
            nc.sync.dma_start(ah0, ahT[0:P, :])
            ah1 = res.tile([P, NSH], bf16, name="ah1")
            nc.sync.dma_start(ah1, ahT[P:NA, :])
            bnw_c = pro.tile([FP, P], fp32)
            nc.sync.dma_start(bnw_c, bnw16)
            bnb_c = pro.tile([FP, P], fp32)
            nc.sync.dma_start(bnb_c, bnb16)
            psts = []
            for rt in range(RT):
                pst = psb.tile([P, F], fp16, name="pst")
                nc.sync.dma_start(pst, ps_t[rt])
                psts.append(pst)

            # ---------------- phase 2: S1/S2 partials ----------------
            # H = G @ W^T in single bf16; S2 = colsum(H*W) via an fp32 ones
            # pass directly on qf (no bf16 eviction hop); S1 = colsum.W^T.
            srow = pro.tile([1, 2 * F], fp32)   # cols 0:F = S1 partial, F:2F = S2
            with tc.tile_pool(name=f"pro2{it}", bufs=1, space="PSUM") as pp2, \
                 tc.tile_pool(name=f"qtmp{it}", bufs=2) as qtmp:
                qfs = []
                def s2_pass(fc):
                    q0, q1 = qfs[fc]
                    fsl = ts(fc, FCW)
                    ps2 = pp2.tile([1, FCW], fp32, name="ps2", tag="ps2", bufs=2)
                    nc.tensor.matmul(ps2, ones_colb, q0, start=True, stop=False)
                    nc.tensor.matmul(ps2, ones_colb, q1, start=False, stop=True)
                    ps1 = pp2.tile([1, FCW], fp32, name="ps1", tag="ps1", bufs=2)
                    nc.tensor.matmul(ps1, sc0h, w0hr[:, fsl], start=True, stop=False)
                    nc.tensor.matmul(ps1, sc1h, w1hr[:, fsl], start=False, stop=True)
                    # (gpsimd cannot read PSUM on real HW; Act does these)
                    nc.scalar.copy(srow[0:1, fsl], ps1)
                    nc.scalar.copy(srow[0:1, ts(FC + fc, FCW)], ps2)

                for fc in range(FC):
                    fsl = ts(fc, FCW)
                    ph0 = pp2.tile([P, FCW], fp32, name="ph0", tag="ph0", bufs=2)
                    nc.tensor.matmul(ph0, g0h[:, 0:P], w0hr[:, fsl], start=True, stop=False)
                    nc.tensor.matmul(ph0, g1h[:, 0:P], w1hr[:, fsl], start=False, stop=True)
                    ph1 = pp2.tile([P, FCW], fp32, name="ph1", tag="ph1", bufs=2)
                    nc.tensor.matmul(ph1, g0h[:, P:NA], w0hr[:, fsl], start=True, stop=False)
                    nc.tensor.matmul(ph1, g1h[:, P:NA], w1hr[:, fsl], start=False, stop=True)
                    # DVE writes qf as bf16 directly: same rounding the old
                    # Act-copy hop applied, zero extra cost, 8 fewer Act ops
                    qf0 = qtmp.tile([P, FCW], bf16, name="qf0")
                    nc.vector.tensor_tensor(qf0, ph0, w0hr[:, fsl], Alu.mult)
                    qf1 = qtmp.tile([P, FCW], bf16, name="qf1")
                    nc.vector.tensor_tensor(qf1, ph1, w1hr[:, fsl], Alu.mult)
                    qfs.append((qf0, qf1))
                    if fc >= 1:
                        s2_pass(fc - 1)   # pipeline: PE stays a chunk ahead
                s2_pass(FC - 1)

            # ---------------- phase 3: AllReduce of S1,S2 (16KB) -------------
            # KVAR=noar: timing probe that skips the collective (stats become
            # single-core partials -> WRONG outputs, identical dense work).
            if "noar" not in KVAR:
                cc_in = dram.tile([1, 2 * F], fp32, name="cc_in")
                cc_out = dram.tile([1, 2 * F], fp32, addr_space="Shared", name="cc_out")
                nc.scalar.dma_start(cc_in, srow)
                nc.gpsimd.collective_compute(
                    "AllReduce",
                    Alu.add,
                    replica_groups=[list(range(NCORES))],
                    ins=[cc_in.opt()],
                    outs=[cc_out.opt()],
                )
                cc_r = cc_out.rearrange("o (two p c) -> two (o p) c", two=2, p=FP)
                st_src = [cc_r[0], cc_r[1]]
            else:
                cc_loc = dram.tile([1, 2 * F], fp32, name="cc_loc")
                nc.scalar.dma_start(cc_loc, srow)
                sr = cc_loc.rearrange("o (two p c) -> two (o p) c", two=2, p=FP)
                st_src = [sr[0], sr[1]]

            # ---------------- phase 4: stats math in [16,128] layout ---------
            shsl2 = res.tile([2, F], bf16, name="shsl2")  # rows: s hi, s lo
            ttl1 = res.tile([1, F], bf16, name="ttl1")    # t (bf16 only; |t|~.01)
            ones2 = res.tile([2, P], bf16, name="ones2")
            nc.vector.memset(ones2, 1.0)
            with tc.tile_pool(name=f"smath{it}", bufs=1) as sm:
                st12 = sm.tile([FP, 2 * P], fp32)
                nc.gpsimd.dma_start(st12[:, 0:P], st_src[0])
                nc.gpsimd.dma_start(st12[:, P:2 * P], st_src[1])
                st1 = st12[:, 0:P]
                st2 = st12[:, P:2 * P]
                # go token: unblocks the PE warmup dummies right after the AR
                go = res.tile([1, 1], bf16, name="go")
                nc.gpsimd.tensor_copy(go, st12[0:1, 0:1])
                sq = sm.tile([FP, P], fp32)
                nc.vector.tensor_tensor(sq, st1, st1, Alu.mult)
                # vv0 = S2 - S1^2/N  (= N*(var+eps) - N*eps)
                vv0 = sm.tile([FP, P], fp32)
                nc.vector.scalar_tensor_tensor(vv0, sq, -1.0 / N, st2, Alu.mult, Alu.add)
                # rr = sqrt(vv0 + N*eps): eps rides the activation bias
                epsb = sm.tile([FP, 1], fp32)
                nc.vector.memset(epsb, float(N * BN_EPS))
                rr = sm.tile([FP, P], fp32)
                nc.scalar.activation(rr, vv0, Act.Sqrt, bias=epsb)
                y0 = sm.tile([FP, P], fp32)
                nc.vector.reciprocal(y0, rr)
                # one Newton step for 1/sqrt(vv) (ScalarE Sqrt is low-precision):
                # y = y0*(1.5 - 0.5*vv*y0^2), with -0.5*vv prefolded
                yy = sm.tile([FP, P], fp32)
                nc.vector.tensor_tensor(yy, y0, y0, Alu.mult)
                nvv = sm.tile([FP, P], fp32)
                nc.vector.tensor_scalar(nvv, vv0, -0.5, -0.5 * N * BN_EPS, Alu.mult, Alu.add)
                u = sm.tile([FP, P], fp32)
                nc.vector.tensor_tensor(u, nvv, yy, Alu.mult)
                y = sm.tile([FP, P], fp32)
                nc.vector.scalar_tensor_tensor(y, u, 1.5, y0, Alu.add, Alu.mult)
                # s = sqrt(N) * y * bn_w; folded bias t = bn_b - (S1/N)*s.
                # s splits + gathers first: the W' fold chain (pbf -> w0h ->
                # px) is longer than the bias-row path, so s leaves earliest.
                s_c = sm.tile([FP, P], fp32)
                nc.vector.scalar_tensor_tensor(s_c, y, float(np.sqrt(N)), bnw_c, Alu.mult, Alu.mult)
                sh_c = sm.tile([FP, P], bf16)
                nc.vector.tensor_copy(sh_c, s_c)
                sl_c = sm.tile([FP, P], bf16)
                nc.vector.tensor_tensor(sl_c, s_c, sh_c, Alu.subtract)
                nc.gpsimd.dma_start(shsl2[0:1, :], sh_c)
                nc.scalar.dma_start(shsl2[1:2, :], sl_c)
                tm = sm.tile([FP, P], fp32)
                nc.vector.scalar_tensor_tensor(tm, st1, -1.0 / N, s_c, Alu.mult, Alu.mult)
                t_c = sm.tile([FP, P], fp32)
                nc.vector.tensor_tensor(t_c, tm, bnb_c, Alu.add)
                th_c = sm.tile([FP, P], bf16)
                nc.vector.tensor_copy(th_c, t_c)
                nc.sync.dma_start(ttl1, th_c)

            # ---------------- phase 5: fold scale into W^T (bf16 hi only) ----
            # PE warmup: the p-state ramp needs ~3us of continuous busy for
            # full clock. These dummy K=1 passes are gated on the AR result
            # (via the Pool-written go token) and run during the stats-math /
            # gather window, so pb/w0h/px hit the ramped clock.
            w0h = res.tile([P, F], bf16, name="w0h")
            w1h = res.tile([P, F], bf16, name="w1h")
            with tc.tile_pool(name=f"pro3{it}", bufs=2, space="PSUM") as pp3, \
                 tc.tile_pool(name=f"wsc{it}", bufs=2) as wsc:
                trash = pp3.tile([1, FCW], fp32, name="trash", tag="trash", bufs=1)
                for _ in range(8):
                    nc.tensor.matmul(trash, go, w0hr[0:1, 0:FCW], start=True, stop=True)
                # full-width s broadcast (PSUM), then ONE fold op per W half,
                # split DVE/Pool so they run in parallel
                pbf = pp3.tile([P, F], fp32, name="pbf", tag="pbf", bufs=1)
                for fc in range(FC):
                    nc.tensor.matmul(pbf[:, ts(fc, FCW)], ones2, shsl2[:, ts(fc, FCW)],
                                     start=True, stop=True)
                # both on DVE (gpsimd cannot read PSUM); px pass1 only needs
                # w0h, so the w1h fold overlaps the first px chunks
                nc.vector.tensor_tensor(w0h, w0hr, pbf, Alu.mult)
                nc.vector.tensor_tensor(w1h, w1hr, pbf, Alu.mult)
                # bridge the fold window too: any PE idle gap resets the ramp
                for _ in range(12):
                    nc.tensor.matmul(trash, go, w0hr[0:1, 0:FCW], start=True, stop=True)

            # ---------------- main loop over 16 row-tiles ----------------
            ones1 = res.tile([1, P], bf16, name="ones1")
            nc.vector.memset(ones1, 1.0)
            with tc.tile_pool(name=f"mx{it}", bufs=8, space="PSUM") as mxp, \
                 tc.tile_pool(name=f"zb{it}", bufs=2) as zb, \
                 tc.tile_pool(name=f"mb{it}", bufs=3) as mb, \
                 tc.tile_pool(name=f"ub{it}", bufs=3) as ub, \
                 tc.tile_pool(name=f"nb{it}", bufs=3) as nb, \
                 tc.tile_pool(name=f"rsb{it}", bufs=4) as rsb:
                for rt in range(RT):
                    rsl = ts(rt, P)
                    pst = psts[rt]   # psn = -ps (negated on host)
                    px = mxp.tile([P, F], fp32, name="px", tag="px", bufs=2)
                    ptypes = [(ah0[:, rsl], w0h), (ah1[:, rsl], w1h),
                              (ones1, ttl1)]
                    for pi, (lhsT, rhs) in enumerate(ptypes):
                        for fc in range(FC):
                            nc.tensor.matmul(px[:, ts(fc, FCW)], lhsT, rhs[:, ts(fc, FCW)],
                                             start=(pi == 0), stop=(pi == len(ptypes) - 1))
                    # zt = xn*psn = -z; rs = rowsum(zt) = -sum(z)
                    zt = zb.tile([P, F], fp32, name="zt")
                    rs = rsb.tile([P, 1], fp32, name="rs")
                    nc.vector.scalar_tensor_tensor(
                        zt, px, 1.0, pst, Alu.mult, Alu.mult, accum_out=rs,
                    )
                    # tau = (sum(z)+1)/2047; ntau = -tau
                    ntau = rsb.tile([P, 1], fp32, name="ntau")
                    nc.vector.tensor_scalar(ntau, rs, INV_D1, -INV_D1, Alu.mult, Alu.add)
                    # m = relu(z - tau) = relu(-zt + ntau)  [fp16 out]
                    mt = mb.tile([P, F], fp16, name="mt")
                    nc.scalar.activation(mt, zt, Act.Relu, bias=ntau, scale=-1.0)
                    if rt == RT - 1:
                        # Act is idle after the last mt; keeps SP free for the
                        # final nps half so the two tails drain in parallel
                        nc.scalar.dma_start(m_t[rt], mt)
                    else:
                        nc.sync.dma_start(m_t[rt], mt)
                    # nps = ps*(GAMMA - m) = (m - GAMMA)*psn.
                    # (Pool only supports TensorTensor, so the -GAMMA shift
                    # runs elsewhere: DVE 16-bit mode is cheapest (594ns) but
                    # DVE is the steady cap, so ~6/16 tiles use Act's idle.)
                    ut = ub.tile([P, F], fp16, name="ut")
                    nc.vector.tensor_scalar_sub(ut, mt, GAMMA)
                    nt = nb.tile([P, F], fp16, name="nt")
                    if rt == RT - 1:
                        # final tile: halve nt across Pool and DVE so the
                        # drain's last product isn't serial on one engine
                        nc.gpsimd.tensor_tensor(nt[:, 0:F // 2], ut[:, 0:F // 2],
                                                pst[:, 0:F // 2], Alu.mult)
                        nc.vector.tensor_tensor(nt[:, F // 2:F], ut[:, F // 2:F],
                                                pst[:, F // 2:F], Alu.mult)
                    else:
                        nc.gpsimd.tensor_tensor(nt, ut, pst, Alu.mult)
                    # parity split balances SP/Pool; swap for the final pair so
                    # the tail m15/nps15 writes land on different queues, and
                    # split the very last nps across both queues to halve the
                    # drain's final DMA
                    if rt == RT - 1:
                        nc.gpsimd.dma_start(nps_t[rt][:, 0:F // 2], nt[:, 0:F // 2])
                        nc.sync.dma_start(nps_t[rt][:, F // 2:F], nt[:, F // 2:F])
                    elif (rt % 2 == 0 if rt < RT - 2 else rt % 2 == 1):
                        nc.gpsimd.dma_start(nps_t[rt], nt)
                    else:
                        nc.sync.dma_start(nps_t[rt], nt)
            pro.release()
            psb.release()
            if "bar" in KVAR and it < KLOOP - 1:
                # cross-iteration serializer for K-loop timing probes: a tiny
                # AllReduce whose input depends on the last output tile
                bin_ = dram.tile([1, 8], fp16, name="bar_in")
                bout = dram.tile([1, 8], fp16, addr_space="Shared", name="bar_out")
                nc.sync.dma_start(bin_, nt[0:1, 0:8])
                nc.gpsimd.collective_compute(
                    "AllReduce",
                    Alu.add,
                    replica_groups=[list(range(NCORES))],
                    ins=[bin_.opt()],
                    outs=[bout.opt()],
                )
                bs = res.tile([1, 8], fp16, name="bar_s")
                # on the SP queue: stalls it until the barrier completes, so
                # the next iteration's prefetch cannot start early
                nc.sync.dma_start(bs, bout)

    nc.compile()
    return nc


def _get_nc(kloop=None):
    key = ("nc", KLOOP if kloop is None else kloop)
    if key not in _CACHE:
        _CACHE[key] = _build_bass(kloop=key[1])
    return _CACHE[key]


def _make_in_maps(a, ps, W, b, bn_w, bn_b):
    import ml_dtypes
    a = np.ascontiguousarray(a, dtype=np.float32)
    ah = a.astype(ml_dtypes.bfloat16)
    psn16 = np.ascontiguousarray((-ps).astype(np.float16))
    wTh_np = np.ascontiguousarray(W.astype(np.float32).T.astype(ml_dtypes.bfloat16))
    bnw16 = np.ascontiguousarray(bn_w.astype(np.float32).reshape(FP, P))
    bnb16 = np.ascontiguousarray(bn_b.astype(np.float32).reshape(FP, P))
    in_maps = []
    for c in range(NCORES):
        rows = slice(c * NSH, (c + 1) * NSH)
        ah_c = ah[rows]
        ah_aug = np.concatenate([ah_c, np.ones((NSH, 1), ah.dtype)], axis=1)
        in_maps.append({
            "ah_aug": np.ascontiguousarray(ah_aug),
            "ahT": np.ascontiguousarray(ah_c.T),
            "wTh": wTh_np,
            "ps_in": np.ascontiguousarray(psn16[rows]),
            "bnw16": bnw16,
            "bnb16": bnb16,
        })
    return in_maps


def run(a, ps, W, b, bn_w, bn_b, trace=False, **kw):
    """Run the kernel on the 8 NeuronCores; returns ((m, new_ps), BassKernelResults)."""
    from concourse import bass_utils

    nc = _get_nc()
    in_maps = _make_in_maps(a, ps, W, b, bn_w, bn_b)
    res = bass_utils.run_bass_kernel_spmd(
        nc, in_maps, core_ids=list(range(NCORES)), trace=trace, **kw,
    )
    m = np.concatenate([r["m_out"] for r in res.results], axis=0).astype(np.float32)
    nps = np.concatenate([r["nps_out"] for r in res.results], axis=0).astype(np.float32)
    return (m, nps), res


def kernel(a, ps, W, b, bn_w, bn_b):
    (m, nps), _ = run(a, ps, W, b, bn_w, bn_b, trace=False)
    return m, nps


if __name__ == "__main__":
    rng = np.random.default_rng(0)
    a = rng.standard_normal((N, NA), dtype=np.float32)
    ps = rng.random((N, F), dtype=np.float32)
    lim = 1.0 / np.sqrt(NA)
    W = rng.uniform(-lim, lim, (F, NA)).astype(np.float32)
    b = rng.uniform(-lim, lim, (F,)).astype(np.float32)
    bn_w = np.ones((F,), np.float32)
    bn_b = np.zeros((F,), np.float32)
    (m, nps), res = run(a, ps, W, b, bn_w, bn_b)
    print("m", m.shape, m.dtype, "nps", nps.shape)
    print("exec_time_ns:", res.exec_time_ns)
